# revision 16
# baseline (speedup 1.0000x reference)
"""Trainium2 Bass kernel for nn_EnhancedTransformerLayer (RoPE attention + MoE).

Sharding: 8 cores; core c -> batch b=c//4, query chunk qc=c%4 (512 contiguous
query tokens). Each core recomputes K/V for its whole batch (dense causal
attention with data-driven masks so all 8 cores share one NEFF), then dense
8-expert MoE on its 512 tokens with top-2 combine. Activations/weights are
float32r (full-rate TensorE, ~1.5e-4 matmul rel err); expert weights bf16
(post-routing value path only). Host pre-transposes/permutes weights so every
DMA is contiguous.

Layouts (all transposed, features on partitions):
  xt  [E, S]   x[b].T with perm rows (per head: even-idx feats, then odd)
  kT  [128, 8, S]   K.T by dout chunk;  qT likewise [128, 8, 512]
  V   [S, E] standard (DRAM scratch) for ctx lhsT
  scores_T [k, q] per (head, kchunk); softmax denom via ones-matmul
"""
import sys, os
sys.path.insert(0, '/opt/trn_rl_repo')
import numpy as np
import ml_dtypes

import concourse.bass as bass
from concourse import bacc
import concourse.tile as tile
from concourse import mybir
from concourse.bass_utils import run_bass_kernel_spmd

R = mybir.dt.float32r
F = mybir.dt.float32
BF = mybir.dt.bfloat16
P = 128
B, S, E, H, D, NE = 2, 2048, 1024, 16, 64, 8
NC = E // P          # 8 feature chunks
QL = 512             # query tokens per core
KC = S // P          # 16 key chunks
EXP_SCALE = 1.0 / (D ** 0.5)
LN_EPS = 1e-5

_cache = {}


def _build():
    nc = bacc.Bacc("TRN2", target_bir_lowering=False, debug=False, num_devices=8)

    def din(name, shape, dt=R):
        return nc.dram_tensor(name, shape, dt, kind="ExternalInput")

    xt = din("xt", [E, S])
    xtq = din("xtq", [E, QL])              # x[b,:,qsl].T perm rows (per-core)
    xres = din("xres", [E, QL])            # x[b,:,qsl].T unpermuted
    wq = din("wq", [E, E]); wk = din("wk", [E, E]); wv = din("wv", [E, E])
    bq = din("bq", [P, NC], F); bk = din("bk", [P, NC], F)
    bvr = din("bvr", [1, E])
    wo = din("wo", [E, E]); bo = din("bo", [P, NC], F)
    gw = din("gw", [E, NE]); gb = din("gb", [NE, 1], F)
    cos2 = din("cos2", [P, S]); sin2 = din("sin2", [P, S])
    cos2q = din("cos2q", [P, QL]); sin2q = din("sin2q", [P, QL])
    maskd = din("maskd", [KC, P, QL])
    ew = din("ew", [NE, NC, P, E], BF)
    ebr = din("ebr", [P, NE * NC], F)
    ln1w = din("ln1w", [P, NC], F); ln1b = din("ln1b", [P, NC], F)
    ln2w = din("ln2w", [P, NC], F); ln2b = din("ln2b", [P, NC], F)
    out = nc.dram_tensor("out", [E, QL], R, kind="ExternalOutput")
    vsc = nc.dram_tensor("vsc", [S, E], R)
    xrd = nc.dram_tensor("xrd", [E, S], R)
    xrqd = nc.dram_tensor("xrqd", [E, QL], R)

    AX = mybir.AxisListType.X
    OP = mybir.AluOpType
    AF = mybir.ActivationFunctionType
    from concourse.masks import make_identity
    import contextlib

    def rope(dst, src, cos_sb, sin_sb, tmppool, width):
        """dst/src [P, NC, width]; chunks 0-3 <- r1, 4-7 <- r2."""
        for c in range(4):
            t1 = tmppool.tile([P, width], R, tag="ropet1")
            t2 = tmppool.tile([P, width], R, tag="ropet2")
            nc.vector.tensor_tensor(out=dst[:, c, :], in0=src[:, c, :], in1=cos_sb[:], op=OP.mult)
            nc.vector.tensor_tensor(out=t1[:], in0=src[:, c + 4, :], in1=sin_sb[:], op=OP.mult)
            nc.vector.tensor_tensor(out=dst[:, c, :], in0=dst[:, c, :], in1=t1[:], op=OP.subtract)
            nc.vector.tensor_tensor(out=t2[:], in0=src[:, c, :], in1=sin_sb[:], op=OP.mult)
            nc.vector.tensor_tensor(out=dst[:, c + 4, :], in0=src[:, c + 4, :], in1=cos_sb[:], op=OP.mult)
            nc.vector.tensor_tensor(out=dst[:, c + 4, :], in0=dst[:, c + 4, :], in1=t2[:], op=OP.add)

    with tile.TileContext(nc) as tc, \
         nc.allow_low_precision(reason="float32r is bit-identical to float32"), \
         contextlib.ExitStack() as es:
        consts = es.enter_context(tc.tile_pool(name="consts", bufs=1))

        # ---- constants ----
        ones_f = consts.tile([P, 1], F, tag="ones_f")
        nc.vector.memset(ones_f[:], 1.0)
        ones = consts.tile([P, 1], R, tag="ones")
        nc.vector.tensor_copy(out=ones[:], in_=ones_f[:])
        ones1_f = consts.tile([1, P], F, tag="ones1_f")
        nc.vector.memset(ones1_f[:], 1.0)
        ones1 = consts.tile([1, P], R, tag="ones1")
        nc.vector.tensor_copy(out=ones1[:], in_=ones1_f[:])
        iden_f = consts.tile([P, P], F, tag="iden_f")
        make_identity(nc, iden_f[:])
        iden = consts.tile([P, P], R, tag="iden")
        nc.vector.tensor_copy(out=iden[:], in_=iden_f[:])
        eps1 = consts.tile([1, 1], F, tag="eps1")
        nc.vector.memset(eps1[:], LN_EPS)

        bq_sb = consts.tile([P, NC], F, tag="bq"); nc.sync.dma_start(bq_sb[:], bq[:])
        bk_sb = consts.tile([P, NC], F, tag="bk"); nc.sync.dma_start(bk_sb[:], bk[:])
        bv_sb = consts.tile([1, E], R, tag="bv"); nc.sync.dma_start(bv_sb[:], bvr[:])
        bo_sb = consts.tile([P, NC], F, tag="bo"); nc.sync.dma_start(bo_sb[:], bo[:])
        gb_sb = consts.tile([NE, 1], F, tag="gb"); nc.sync.dma_start(gb_sb[:], gb[:])
        gw_sb = consts.tile([P, NC, NE], R, tag="gw")
        nc.sync.dma_start(gw_sb[:], gw.rearrange("(c p) g -> p c g", p=P))
        eb_sb = consts.tile([P, NE * NC], F, tag="eb"); nc.sync.dma_start(eb_sb[:], ebr[:])
        ln_sb = {}
        for nm, t in (("ln1w", ln1w), ("ln1b", ln1b), ("ln2w", ln2w), ("ln2b", ln2b)):
            ln_sb[nm] = consts.tile([P, NC], F, tag=nm, name=nm)
            nc.sync.dma_start(ln_sb[nm][:], t[:])

        # =========== Phase A1: rope -> xrd/xrqd DRAM scratch ===========
        # persistent ctx (written end of B, read in C)
        persist = es.enter_context(tc.tile_pool(name="persist", bufs=1))
        ctx_sb = persist.tile([P, NC, QL], R, tag="ctx")

        xt_r = xt.rearrange("(c p) s -> p c s", p=P)
        xrd_r = xrd.rearrange("(c p) s -> p c s", p=P)
        with tc.tile_pool(name="ropep", bufs=1) as ropep, \
             tc.tile_pool(name="xt2p", bufs=2) as xt2p, \
             tc.tile_pool(name="xrev", bufs=2) as xrev, \
             tc.tile_pool(name="a1tmp", bufs=1) as a1tmp:
            cos_sb = ropep.tile([P, S], R, tag="cos2")
            sin_sb = ropep.tile([P, S], R, tag="sin2")
            nc.sync.dma_start(cos_sb[:], cos2[:])
            nc.sync.dma_start(sin_sb[:], sin2[:])
            for c in range(4):
                xt2 = xt2p.tile([P, 2, S], R, tag="xt2")
                nc.sync.dma_start(xt2[:, 0, :], xt_r[:, c, :])
                nc.sync.dma_start(xt2[:, 1, :], xt_r[:, c + 4, :])
                t1 = a1tmp.tile([P, S], R, tag="ropet1")
                t2 = a1tmp.tile([P, S], R, tag="ropet2")
                xo = xrev.tile([P, 2, S], R, tag="xo")
                nc.vector.tensor_tensor(out=xo[:, 0, :], in0=xt2[:, 0, :], in1=cos_sb[:], op=OP.mult)
                nc.vector.tensor_tensor(out=t1[:], in0=xt2[:, 1, :], in1=sin_sb[:], op=OP.mult)
                nc.vector.tensor_tensor(out=xo[:, 0, :], in0=xo[:, 0, :], in1=t1[:], op=OP.subtract)
                nc.vector.tensor_tensor(out=t2[:], in0=xt2[:, 0, :], in1=sin_sb[:], op=OP.mult)
                nc.vector.tensor_tensor(out=xo[:, 1, :], in0=xt2[:, 1, :], in1=cos_sb[:], op=OP.mult)
                nc.vector.tensor_tensor(out=xo[:, 1, :], in0=xo[:, 1, :], in1=t2[:], op=OP.add)
                nc.sync.dma_start(xrd_r[:, c, :], xo[:, 0, :])
                nc.sync.dma_start(xrd_r[:, c + 4, :], xo[:, 1, :])

            # rope of own q chunk (per-core data) -> xrqd
            xtq_sb = ropep.tile([P, NC, QL], R, tag="xtq")
            nc.sync.dma_start(xtq_sb[:], xtq.rearrange("(c p) q -> p c q", p=P))
            cosq_sb = ropep.tile([P, QL], R, tag="cosq")
            sinq_sb = ropep.tile([P, QL], R, tag="sinq")
            nc.sync.dma_start(cosq_sb[:], cos2q[:])
            nc.sync.dma_start(sinq_sb[:], sin2q[:])
            xrq_sb = ropep.tile([P, NC, QL], R, tag="xrq")
            rope(xrq_sb, xtq_sb, cosq_sb, sinq_sb, a1tmp, QL)
            nc.sync.dma_start(xrqd.rearrange("(c p) q -> p c q", p=P), xrq_sb[:])

        # =========== Phase A1b: V projection -> vsc DRAM ===========
        with tc.tile_pool(name="wvp", bufs=1) as wvp, \
             tc.tile_pool(name="xtok", bufs=3) as xtok, \
             tc.tile_pool(name="vev_p", bufs=3) as vev_p, \
             tc.tile_pool(name="vps", bufs=4, space="PSUM") as vps:
            wv_sb = wvp.tile([P, NC, E], R, tag="wv")
            for c in range(NC):
                nc.sync.dma_start(wv_sb[:, c, :],
                                  wv.rearrange("(c p) m -> p c m", p=P)[:, c, :])
            for tkc in range(S // P):
                xt_tok = xtok.tile([P, NC, P], R, tag="xt_tok")
                nc.sync.dma_start(xt_tok[:], xt_r[:, :, tkc * P:(tkc + 1) * P])
                for dvs in range(2):
                    vp = vps.tile([P, 512], F, tag="vps")
                    for dc in range(NC):
                        nc.tensor.matmul(
                            vp[:], xt_tok[:, dc, :],
                            wv_sb[:, dc, dvs * 512:(dvs + 1) * 512],
                            start=(dc == 0), stop=False)
                    nc.tensor.matmul(
                        vp[:], ones1[:, :], bv_sb[:, dvs * 512:(dvs + 1) * 512],
                        start=False, stop=True)
                    vev = vev_p.tile([P, 512], R, tag="vev")
                    nc.vector.tensor_copy(out=vev[:], in_=vp[:])
                    nc.sync.dma_start(
                        vsc[tkc * P:(tkc + 1) * P, dvs * 512:(dvs + 1) * 512], vev[:])

        # =========== Phase A2: K then Q projections ===========
        attn_res_cm = tc.tile_pool(name="attn_res", bufs=1)
        attn_res = attn_res_cm.__enter__()
        kT = attn_res.tile([P, NC, S], R, tag="kT")
        qT = attn_res.tile([P, NC, QL], R, tag="qT")
        with tc.tile_pool(name="wkp", bufs=1) as wkp, \
             tc.tile_pool(name="xrsp", bufs=2) as xrsp, \
             tc.tile_pool(name="kqps", bufs=4, space="PSUM") as kqps:
            wk_sb = wkp.tile([P, NC, E], R, tag="wk_sb")
            for c in range(NC):
                nc.sync.dma_start(wk_sb[:, c, :],
                                  wk.rearrange("(c p) m -> p c m", p=P)[:, c, :])
            for sp in range(S // 512):
                xr_sp = xrsp.tile([P, NC, 512], R, tag="xr_sp")
                nc.sync.dma_start(xr_sp[:], xrd_r[:, :, sp * 512:(sp + 1) * 512])
                for oc in range(NC):
                    kp = kqps.tile([P, 512], F, tag="kps")
                    for dc in range(NC):
                        nc.tensor.matmul(
                            kp[:], wk_sb[:, dc, oc * P:(oc + 1) * P], xr_sp[:, dc, :],
                            start=(dc == 0), stop=(dc == NC - 1))
                    nc.vector.tensor_scalar(
                        out=kT[:, oc, sp * 512:(sp + 1) * 512], in0=kp[:],
                        scalar1=bk_sb[:, oc:oc + 1], scalar2=None, op0=OP.add)
        with tc.tile_pool(name="wqp", bufs=1) as wqp, \
             tc.tile_pool(name="xrqp2", bufs=1) as xrqp2, \
             tc.tile_pool(name="qps_p", bufs=4, space="PSUM") as qps_p:
            wq_sb = wqp.tile([P, NC, E], R, tag="wq_sb")
            for c in range(NC):
                nc.sync.dma_start(wq_sb[:, c, :],
                                  wq.rearrange("(c p) m -> p c m", p=P)[:, c, :])
            xrq2 = xrqp2.tile([P, NC, QL], R, tag="xrq2")
            nc.sync.dma_start(xrq2[:], xrqd.rearrange("(c p) q -> p c q", p=P))
            for oc in range(NC):
                qp = qps_p.tile([P, 512], F, tag="qps")
                for dc in range(NC):
                    nc.tensor.matmul(
                        qp[:], wq_sb[:, dc, oc * P:(oc + 1) * P], xrq2[:, dc, :],
                        start=(dc == 0), stop=(dc == NC - 1))
                nc.vector.tensor_scalar(
                    out=qT[:, oc, :], in0=qp[:],
                    scalar1=bq_sb[:, oc:oc + 1], scalar2=None, op0=OP.add)

        # =========== Phase B: attention ===========
        xres_sb = consts.tile([P, NC, QL], R, tag="xres")
        nc.sync.dma_start(xres_sb[:], xres.rearrange("(c p) q -> p c q", p=P))
        with tc.tile_pool(name="maskp", bufs=1) as maskp, \
             tc.tile_pool(name="bt", bufs=3) as bt, \
             tc.tile_pool(name="vt_p", bufs=2) as vt_p, \
             tc.tile_pool(name="scps", bufs=3, space="PSUM") as scps, \
             tc.tile_pool(name="ctxps", bufs=2, space="PSUM") as ctxps, \
             tc.tile_pool(name="rowps", bufs=2, space="PSUM") as rowps:
            mask_sb = maskp.tile([P, KC, QL], R, tag="mask")
            nc.sync.dma_start(mask_sb[:], maskd.rearrange("k p q -> p k q"))
            for hp in range(NC):
                vt = vt_p.tile([P, KC, P], R, tag="vt")
                nc.sync.dma_start(
                    vt[:], vsc.rearrange("(k p) d -> p k d", p=P)[:, :, hp * P:(hp + 1) * P])
                for hh in range(2):
                    ctxp = ctxps.tile([64, QL], F, tag="ctxps")
                    denp = rowps.tile([1, QL], F, tag="denps")
                    for kc in range(KC):
                        scp = scps.tile([P, QL], F, tag="scps")
                        nc.tensor.matmul(
                            scp[:], kT[hh * 64:(hh + 1) * 64, hp, kc * P:(kc + 1) * P],
                            qT[hh * 64:(hh + 1) * 64, hp, :], start=True, stop=True)
                        st = bt.tile([P, QL], R, tag="st")
                        nc.scalar.activation(out=st[:], in_=scp[:], func=AF.Exp, scale=EXP_SCALE)
                        nc.vector.tensor_tensor(out=st[:], in0=st[:], in1=mask_sb[:, kc, :], op=OP.mult)
                        nc.tensor.matmul(denp[:], ones[:], st[:],
                                         start=(kc == 0), stop=(kc == KC - 1))
                        nc.tensor.matmul(ctxp[:], vt[:, kc, hh * 64:(hh + 1) * 64], st[:],
                                         start=(kc == 0), stop=(kc == KC - 1))
                    rden = bt.tile([1, QL], R, tag="rden")
                    nc.vector.reciprocal(out=rden[:], in_=denp[:])
                    rbp = scps.tile([P, QL], F, tag="scps")
                    nc.tensor.matmul(rbp[0:64, :], ones1[:, 0:64], rden[:], start=True, stop=True)
                    rb_sb = bt.tile([64, QL], R, tag="rb_sb")
                    nc.scalar.copy(out=rb_sb[:], in_=rbp[0:64, :])
                    nc.vector.tensor_tensor(
                        out=ctx_sb[hh * 64:(hh + 1) * 64, hp, :],
                        in0=ctxp[:], in1=rb_sb[:], op=OP.mult)
        attn_res_cm.__exit__(None, None, None)  # free kT/qT

        # =========== helpers for LN ===========
        def layernorm(src, dst, wtile, btile, tmp, ps_row, ps_big):
            sp_ = ps_row.tile([1, QL], F, tag="lnsum")
            for c in range(NC):
                nc.tensor.matmul(sp_[:], ones[:], src[:, c, :],
                                 start=(c == 0), stop=(c == NC - 1))
            s2p = ps_row.tile([1, QL], F, tag="lnsum2")
            for c in range(NC):
                sq = tmp.tile([P, QL], R, tag="lnsq")
                nc.vector.tensor_tensor(out=sq[:], in0=src[:, c, :], in1=src[:, c, :], op=OP.mult)
                nc.tensor.matmul(s2p[:], ones[:], sq[:],
                                 start=(c == 0), stop=(c == NC - 1))
            mean = tmp.tile([1, QL], R, tag="lnmean")
            nc.scalar.mul(out=mean[:], in_=sp_[:], mul=1.0 / E)
            msq = tmp.tile([1, QL], R, tag="lnmsq")
            nc.scalar.mul(out=msq[:], in_=s2p[:], mul=1.0 / E)
            var = tmp.tile([1, QL], R, tag="lnvar")
            nc.vector.tensor_tensor(out=var[:], in0=mean[:], in1=mean[:], op=OP.mult)
            nc.vector.tensor_tensor(out=var[:], in0=msq[:], in1=var[:], op=OP.subtract)
            std = tmp.tile([1, QL], R, tag="lnstd")
            nc.scalar.activation(out=std[:], in_=var[:], func=AF.Sqrt, bias=eps1[:])
            rstd = tmp.tile([1, QL], R, tag="lnrstd")
            nc.vector.reciprocal(out=rstd[:], in_=std[:])
            mb = ps_big.tile([P, QL], F, tag="bigc")
            nc.tensor.matmul(mb[:], ones1[:], mean[:], start=True, stop=True)
            rb = ps_big.tile([P, QL], F, tag="bigc")
            nc.tensor.matmul(rb[:], ones1[:], rstd[:], start=True, stop=True)
            for c in range(NC):
                t = tmp.tile([P, QL], R, tag="lnt")
                nc.vector.tensor_tensor(out=t[:], in0=src[:, c, :], in1=mb[:], op=OP.subtract)
                nc.vector.tensor_tensor(out=t[:], in0=t[:], in1=rb[:], op=OP.mult)
                nc.vector.tensor_scalar(out=dst[:, c, :], in0=t[:],
                                        scalar1=wtile[:, c:c + 1], scalar2=btile[:, c:c + 1],
                                        op0=OP.mult, op1=OP.add)

        # =========== Phase C: out-proj + LN1 + gates ===========
        cres = es.enter_context(tc.tile_pool(name="cres", bufs=1))
        x1 = cres.tile([P, NC, QL], R, tag="x1")
        wbc = cres.tile([P, NE, QL], R, tag="wbc")
        x1b = cres.tile([P, NC, QL], BF, tag="x1b")
        with tc.tile_pool(name="cslab", bufs=2) as cslab, \
             tc.tile_pool(name="ct", bufs=2) as ct, \
             tc.tile_pool(name="h1p", bufs=1) as h1p, \
             tc.tile_pool(name="cps", bufs=3, space="PSUM") as cps, \
             tc.tile_pool(name="crow", bufs=1, space="PSUM") as crow, \
             tc.tile_pool(name="cg", bufs=1, space="PSUM") as cg:
            h1 = h1p.tile([P, NC, QL], R, tag="h1")
            for oc in range(NC):
                wo_sl = cslab.tile([P, NC, P], R, tag="wo_sl")
                nc.sync.dma_start(
                    wo_sl[:], wo.rearrange("(c p) m -> p c m", p=P)[:, :, oc * P:(oc + 1) * P])
                ap = cps.tile([P, QL], F, tag="bigc")
                for dc in range(NC):
                    nc.tensor.matmul(ap[:], wo_sl[:, dc, :], ctx_sb[:, dc, :],
                                     start=(dc == 0), stop=(dc == NC - 1))
                nc.vector.tensor_scalar(out=h1[:, oc, :], in0=ap[:],
                                        scalar1=bo_sb[:, oc:oc + 1], scalar2=None, op0=OP.add)
                nc.vector.tensor_tensor(out=h1[:, oc, :], in0=h1[:, oc, :],
                                        in1=xres_sb[:, oc, :], op=OP.add)

            layernorm(h1, x1, ln_sb["ln1w"], ln_sb["ln1b"], ct, crow, cps)

            # gates
            gp = cg.tile([NE, QL], F, tag="gps")
            for c in range(NC):
                nc.tensor.matmul(gp[:], gw_sb[:, c, :], x1[:, c, :],
                                 start=(c == 0), stop=(c == NC - 1))
            glog = ct.tile([NE, QL], R, tag="glog")
            nc.vector.tensor_scalar(out=glog[:], in0=gp[:], scalar1=gb_sb[:],
                                    scalar2=None, op0=OP.add)
            gexp = ct.tile([NE, QL], R, tag="gexp")
            nc.scalar.activation(out=gexp[:], in_=glog[:], func=AF.Exp)
            dgp = crow.tile([1, QL], F, tag="lnsum")
            nc.tensor.matmul(dgp[:], ones[0:NE, :], gexp[:], start=True, stop=True)
            rg = ct.tile([1, QL], R, tag="rg")
            nc.vector.reciprocal(out=rg[:], in_=dgp[:])

            # top-2 via transposes
            gt = ct.tile([P, 4, NE], R, tag="gt")
            for qb in range(4):
                gtp = cg.tile([P, NE], R, tag="gmix")
                nc.tensor.transpose(gtp[:], gexp[:, qb * P:(qb + 1) * P], iden[0:NE, 0:NE])
                nc.vector.tensor_copy(out=gt[:, qb, :], in_=gtp[:])
            m1 = ct.tile([P, 4], F, tag="m1")
            nc.vector.reduce_max(out=m1[:], in_=gt[:], axis=AX)
            msel = ct.tile([P, 4, NE], R, tag="msel")
            g2 = ct.tile([P, 4, NE], R, tag="g2")
            for qb in range(4):
                nc.vector.tensor_scalar(out=msel[:, qb, :], in0=gt[:, qb, :],
                                        scalar1=m1[:, qb:qb + 1], scalar2=None, op0=OP.is_equal)
            nc.vector.tensor_tensor(out=g2[:], in0=gt[:], in1=msel[:], op=OP.mult)
            nc.vector.tensor_tensor(out=g2[:], in0=gt[:], in1=g2[:], op=OP.subtract)
            m2 = ct.tile([P, 4], F, tag="m2")
            nc.vector.reduce_max(out=m2[:], in_=g2[:], axis=AX)
            msel2 = ct.tile([P, 4, NE], R, tag="msel2")
            for qb in range(4):
                nc.vector.tensor_scalar(out=msel2[:, qb, :], in0=g2[:, qb, :],
                                        scalar1=m2[:, qb:qb + 1], scalar2=None, op0=OP.is_equal)
            nc.vector.tensor_tensor(out=msel[:], in0=msel[:], in1=msel2[:], op=OP.add)
            wsel = ct.tile([P, 4, NE], R, tag="wsel")
            nc.vector.tensor_tensor(out=wsel[:], in0=gt[:], in1=msel[:], op=OP.mult)

            wrow = cres.tile([1, NE, QL], R, tag="wrow")
            for qb in range(4):
                for e in range(NE):
                    wtp = cg.tile([1, P], R, tag="gmix")
                    nc.tensor.transpose(wtp[:], wsel[:, qb, e:e + 1], iden[:])
                    nc.vector.tensor_copy(out=wrow[:, e, qb * P:(qb + 1) * P],
                                          in_=wtp[:])
            for e in range(NE):
                nc.vector.tensor_tensor(out=wrow[:, e, :], in0=wrow[:, e, :],
                                        in1=rg[:], op=OP.mult)
                bcp = cps.tile([P, QL], F, tag="bigc")
                nc.tensor.matmul(bcp[:], ones1[:], wrow[:, e, :], start=True, stop=True)
                nc.vector.tensor_copy(out=wbc[:, e, :], in_=bcp[:])

            for c in range(NC):
                nc.vector.tensor_copy(out=x1b[:, c, :], in_=x1[:, c, :])

        # =========== Phase D: dense MoE ===========
        moe = cres.tile([P, NC, QL], R, tag="moe")
        with tc.tile_pool(name="ewp", bufs=2) as ewp, \
             tc.tile_pool(name="dt", bufs=3) as dt_, \
             tc.tile_pool(name="dps", bufs=4, space="PSUM") as dps:
            for e in range(NE):
                ew_sl = ewp.tile([P, NC, E], BF, tag="ew_sl")
                nc.sync.dma_start(ew_sl[:], ew[e].rearrange("c p d -> p c d"))
                for oc in range(NC):
                    yp = dps.tile([P, QL], F, tag="yps")
                    for dc in range(NC):
                        nc.tensor.matmul(
                            yp[:], ew_sl[:, dc, oc * P:(oc + 1) * P], x1b[:, dc, :],
                            start=(dc == 0), stop=(dc == NC - 1))
                    t = dt_.tile([P, QL], R, tag="moet")
                    nc.vector.tensor_scalar(out=t[:], in0=yp[:],
                                            scalar1=eb_sb[:, e * NC + oc:e * NC + oc + 1],
                                            scalar2=None, op0=OP.add)
                    nc.vector.tensor_tensor(out=t[:], in0=t[:], in1=wbc[:, e, :], op=OP.mult)
                    if e == 0:
                        nc.vector.tensor_copy(out=moe[:, oc, :], in_=t[:])
                    else:
                        nc.vector.tensor_tensor(out=moe[:, oc, :], in0=moe[:, oc, :],
                                                in1=t[:], op=OP.add)

        # =========== Phase E: residual + LN2 + store ===========
        with tc.tile_pool(name="et", bufs=3) as et, \
             tc.tile_pool(name="eps_", bufs=2, space="PSUM") as eps_, \
             tc.tile_pool(name="erow", bufs=1, space="PSUM") as erow:
            for c in range(NC):
                nc.vector.tensor_tensor(out=moe[:, c, :], in0=x1[:, c, :],
                                        in1=moe[:, c, :], op=OP.add)
            layernorm(moe, moe, ln_sb["ln2w"], ln_sb["ln2b"], et, erow, eps_)
            for c in range(NC):
                nc.sync.dma_start(out.rearrange("(c p) q -> p c q", p=P)[:, c, :], moe[:, c, :])

    nc.compile()
    return nc


def _prep_inputs(inputs):
    x = np.asarray(inputs['x'], dtype=np.float32)
    ipw = np.asarray(inputs['in_proj_w'], dtype=np.float32)
    ipb = np.asarray(inputs['in_proj_b'], dtype=np.float32)
    opw = np.asarray(inputs['out_proj_w'], dtype=np.float32)
    opb = np.asarray(inputs['out_proj_b'], dtype=np.float32)
    gww = np.asarray(inputs['gate_w'], dtype=np.float32)
    gbb = np.asarray(inputs['gate_b'], dtype=np.float32)
    eww = np.asarray(inputs['expert_w'], dtype=np.float32)
    ebb = np.asarray(inputs['expert_b'], dtype=np.float32)

    perm = np.empty(E, dtype=np.int64)
    idx = 0
    for h in range(H):
        for i in range(D // 2):
            perm[idx] = 64 * h + 2 * i; idx += 1
    for h in range(H):
        for i in range(D // 2):
            perm[idx] = 64 * h + 2 * i + 1; idx += 1

    Wq, Wk, Wv = ipw[0:E], ipw[E:2 * E], ipw[2 * E:3 * E]
    bq_, bk_, bv_ = ipb[0:E], ipb[E:2 * E], ipb[2 * E:3 * E]
    common = {
        "wq": np.ascontiguousarray(Wq[:, perm].T),
        "wk": np.ascontiguousarray(Wk[:, perm].T),
        "wv": np.ascontiguousarray(Wv[:, perm].T),
        "bq": np.ascontiguousarray(bq_.reshape(NC, P).T),
        "bk": np.ascontiguousarray(bk_.reshape(NC, P).T),
        "bvr": bv_.reshape(1, E).copy(),
        "wo": np.ascontiguousarray(opw.T),
        "bo": np.ascontiguousarray(opb.reshape(NC, P).T),
        "gw": np.ascontiguousarray(gww.T),
        "gb": gbb.reshape(NE, 1).copy(),
        "ew": np.ascontiguousarray(
            eww.transpose(0, 2, 1).reshape(NE, NC, P, E)).astype(ml_dtypes.bfloat16),
        "ebr": np.ascontiguousarray(ebb.reshape(NE, NC, P).transpose(2, 0, 1).reshape(P, NE * NC)),
        "ln1w": np.ascontiguousarray(np.asarray(inputs['ln1_w'], np.float32).reshape(NC, P).T),
        "ln1b": np.ascontiguousarray(np.asarray(inputs['ln1_b'], np.float32).reshape(NC, P).T),
        "ln2w": np.ascontiguousarray(np.asarray(inputs['ln2_w'], np.float32).reshape(NC, P).T),
        "ln2b": np.ascontiguousarray(np.asarray(inputs['ln2_b'], np.float32).reshape(NC, P).T),
    }
    inv_freq = 1.0 / (10000.0 ** (np.arange(0, D, 2, dtype=np.float64) / D))
    freqs = np.arange(S, dtype=np.float64)[:, None] * inv_freq[None, :]  # [S, 32]
    cos_t = np.cos(freqs).T.astype(np.float32)
    sin_t = np.sin(freqs).T.astype(np.float32)
    cos2 = np.ascontiguousarray(np.tile(cos_t, (4, 1)))
    sin2 = np.ascontiguousarray(np.tile(sin_t, (4, 1)))
    common["cos2"] = cos2
    common["sin2"] = sin2

    in_maps = []
    for c in range(8):
        b, qc = c // 4, c % 4
        qsl = slice(qc * QL, (qc + 1) * QL)
        xtb = np.ascontiguousarray(x[b].T)
        xtp = np.ascontiguousarray(xtb[perm])
        m = dict(common)
        m["xt"] = xtp
        m["xtq"] = np.ascontiguousarray(xtp[:, qsl])
        m["xres"] = np.ascontiguousarray(xtb[:, qsl])
        m["cos2q"] = np.ascontiguousarray(cos2[:, qsl])
        m["sin2q"] = np.ascontiguousarray(sin2[:, qsl])
        qg = np.arange(qc * QL, (qc + 1) * QL)[None, None, :]
        kg = (np.arange(KC)[:, None, None] * P + np.arange(P)[None, :, None])
        m["maskd"] = (qg >= kg).astype(np.float32)
        in_maps.append(m)
    return in_maps


def _ensure_ntff_hook():
    """Register the axon NTFF profile hook if the image's antenv lacks it."""
    import types, importlib
    try:
        from antenv.axon_hooks import get_axon_ntff_profile_hook  # noqa
        return True
    except ImportError:
        pass
    try:
        import antenv
        sys.path.insert(0, '/root/.axon_site')
        from trn_agent_boot.trn_boot import _ntff_profile_via_ctypes
        hook = _ntff_profile_via_ctypes('/opt/axon/libaxon_pjrt.so')
        if hook is None:
            return False
        mod = types.ModuleType('antenv.axon_hooks')
        _state = {'hook': hook}
        mod.set_axon_ntff_profile_hook = lambda h: _state.__setitem__('hook', h)
        mod.get_axon_ntff_profile_hook = lambda: _state['hook']
        sys.modules['antenv.axon_hooks'] = mod
        antenv.axon_hooks = mod
        return True
    except Exception as e:
        print(f"ntff hook setup failed: {e}")
        return False


def kernel(**inputs):
    if "nc" not in _cache:
        _cache["nc"] = _build()
    nc = _cache["nc"]
    in_maps = _prep_inputs(inputs)
    trace = bool(int(os.environ.get("KERNEL_TRACE", "0")))
    kwargs = {}
    if trace and _ensure_ntff_hook():
        kwargs = dict(trace=True, trace_cores=list(range(8)))
    res = run_bass_kernel_spmd(nc, in_maps, core_ids=list(range(8)), **kwargs)
    _cache["last_results"] = res
    out = np.empty((B, S, E), dtype=np.float32)
    for c in range(8):
        b, qc = c // 4, c % 4
        out[b, qc * QL:(qc + 1) * QL, :] = res.results[c]["out"].T
    return out
